# revision 1
# baseline (speedup 1.0000x reference)
"""Bilinear interaction layer (pairwise per-field Linear + gate) on 8 trn2 cores.

out[b, p, :] = (femb[b, i_p] @ W[p].T) * femb[b, j_p]   for the P=C(F,2) field
pairs (i_p, j_p) in itertools.combinations order.  B=4096, F=30, D=128, P=435.

Sharding: data-parallel over batch (4096 -> 512 per core), W replicated.

Per core, pairs are processed in "i-blocks" (the (29-i) pairs sharing first
field i, consecutive in p).  For each i-block and each 128-row batch chunk,
TensorE runs fp32 matmuls with the v_i chunk [d=128, b=128] stationary and up
to 4 pairs' transposed weights [d=128, 4*128] moving (N=512, the fp32 moving
limit), producing PSUM [b=128, 4*128] directly in the natural output layout.
VectorE applies the v_j gate straight out of PSUM into an SBUF staging tile
(fused PSUM-read + multiply + SBUF-write), and the staging tile is DMA'd out
with >= 512B-contiguous rows.

DMA engine assignment: all input loads go through SWDGE (GpSimd) so they can
never queue behind backpressured output stores; output stores alternate
between the two HWDGE rings (SP and ACT).  Per core traffic: 44 MB in (W
28.5 + two embedding layouts 15.3) + 114 MB out.  Measured on HW (marginal
time of an in-NEFF repeat loop): ~0.50 ms/call -- simultaneously at the PE
fp32 limit (480 self-loading fp32 matmuls) and the HBM limit.  Output is
bit-identical to an fp32 jax reference on-device and ~2e-7 Frobenius relative
error vs CPU BLAS.
"""

import os
import sys

import numpy as np

for _p in ("/opt/trn_rl_repo", "/root/.axon_site/_ro/trn_rl_repo"):
    if os.path.isdir(_p) and _p not in sys.path:
        sys.path.append(_p)

import concourse.bacc as bacc
import concourse.tile as tile
from concourse import mybir
from concourse.bass_utils import run_bass_kernel_spmd

B, F, D = 4096, 30, 128
P = F * (F - 1) // 2  # 435
NCORES = 8
BSH = B // NCORES  # 512 batches per core
NCHUNK = BSH // 128  # 4 batch chunks of 128
GROUP = 4  # pairs per matmul -> moving dim 512 (fp32 max)
FD = F * D  # 3840
PD = P * D  # 55680

MODE = "load"  # "load" (natural-layout output) or "eb" ([e,b] compute layout)
TRACE = False
last_results = None  # BassKernelResults of the most recent kernel() call

_cache = {}


def _build(niter=1, mode="load", ftl_bufs=3, mm_dt=None, ps_bufs=None, ablate=None, stg_bufs=4, w_bufs=3, wide=0, out_rings=2):
    nc = bacc.Bacc("TRN2", target_bir_lowering=False, debug=False, num_devices=NCORES)
    if mode != "eb":
        femb_n = nc.declare_dram_parameter("femb_n", [BSH, FD], mybir.dt.float32, isOutput=False)
    if mode in ("load", "eb"):
        femb_t = nc.declare_dram_parameter("femb_t", [FD, BSH], mybir.dt.float32, isOutput=False)
    w_t = nc.declare_dram_parameter("w_t", [D, PD], mybir.dt.float32, isOutput=False)
    if mode != "load":
        eye = nc.declare_dram_parameter("eye", [D, D], mybir.dt.float32, isOutput=False)
    if mode == "eb":
        out = nc.declare_dram_parameter("out", [PD, BSH], mybir.dt.float32, isOutput=True)
    else:
        out = nc.declare_dram_parameter("out", [BSH, PD], mybir.dt.float32, isOutput=True)

    import contextlib

    with tile.TileContext(nc) as tc:
        with (
            tc.tile_pool(name="eye", bufs=1) as eye_pool,
            tc.tile_pool(name="fn", bufs=1) as fn_pool,
            tc.tile_pool(name="ftl", bufs=ftl_bufs) as ftl_pool,
            tc.tile_pool(name="w", bufs=w_bufs) as w_pool,
            tc.tile_pool(name="stg", bufs=stg_bufs) as stg_pool,
            tc.tile_pool(name="ps", bufs=ps_bufs or 6, space="PSUM") as ps_pool,
            tc.tile_pool(name="tr", bufs=2, space="PSUM") as tr_pool,
            tc.For_i(
                0,
                niter,
                1,
                hint_engines=(
                    mybir.EngineType.PE,
                    mybir.EngineType.DVE,
                    mybir.EngineType.Activation,
                    mybir.EngineType.SP,
                ),
            )
            if niter > 1
            else contextlib.nullcontext(),
        ):
            if mode == "eb":
                # [e, b] layout: W stationary, activations moving. All of
                # femb_t stays resident (60 KB/partition); output tensor is
                # [P*D, BSH] so every store is a fully-sequential DRAM block.
                # Host un-transposes the result.
                WIN = 8
                femb_all = fn_pool.tile([128, F * BSH], mybir.dt.float32, tag="fa")
                for f in range(F):
                    nc.gpsimd.dma_start(
                        femb_all[:, f * BSH : (f + 1) * BSH],
                        femb_t[f * D : (f + 1) * D, :],
                    )
                out3 = out.reshape([P, D, BSH])
                p0 = 0
                for i in range(F - 1):
                    s = F - 1 - i
                    w_tile = w_pool.tile([128, s * D], mybir.dt.float32, tag="w")
                    nc.gpsimd.dma_start(w_tile[:], w_t[:, p0 * D : (p0 + s) * D])
                    for w0 in range(0, s, WIN):
                        nw = min(WIN, s - w0)
                        stg_tile = stg_pool.tile(
                            [128, WIN, BSH], mybir.dt.float32, tag="stg"
                        )
                        for k in range(nw):
                            pr = w0 + k
                            j = i + 1 + pr
                            ps = ps_pool.tile([128, BSH], mybir.dt.float32, tag="ps")
                            nc.tensor.matmul(
                                ps[:],
                                w_tile[:, pr * D : (pr + 1) * D],  # [K=d, M=e]
                                femb_all[:, i * BSH : (i + 1) * BSH],  # [K=d, N=b]
                                start=True,
                                stop=True,
                            )
                            nc.vector.tensor_mul(
                                stg_tile[:, k, :],
                                ps[:],
                                femb_all[:, j * BSH : (j + 1) * BSH],
                            )
                        rings = [nc.sync, nc.scalar][:out_rings]
                        out_eng = rings[(p0 + w0) % len(rings)]
                        out_eng.dma_start(
                            out3[p0 + w0 : p0 + w0 + nw, :, :].transpose((1, 0, 2)),
                            stg_tile[:, :nw, :],
                        )
                    p0 += s
            else:
                if mode != "load":
                    eye_tile = eye_pool.tile([D, D], mybir.dt.float32)
                    nc.gpsimd.dma_start(eye_tile[:], eye[:])
                # whole femb shard, natural layout: partition=b (within chunk),
                # free=(field, emb); one tile per batch chunk so consumers only
                # wait on the chunk they need.
                fn_tiles = []
                for c in range(NCHUNK):
                    fnt = fn_pool.tile([128, FD], mybir.dt.float32, tag=f"fn{c}")
                    nc.gpsimd.dma_start(fnt[:], femb_n[c * 128 : (c + 1) * 128, :])
                    fn_tiles.append(fnt)

                p0 = 0
                for i in range(F - 1):
                    s = F - 1 - i  # pairs in this i-block: (i, i+1) .. (i, F-1)
                    # Build v_i in [d, b] layout on-chip: PE transpose-mode
                    # (exact data movement) + ScalarE copy out of PSUM.
                    ftl_tile = ftl_pool.tile([128, BSH], mybir.dt.float32, tag="ftl")
                    if mode == "load":
                        nc.gpsimd.dma_start(ftl_tile[:], femb_t[i * D : (i + 1) * D, :])
                    else:
                        for c in range(NCHUNK):
                            trp = tr_pool.tile([128, 128], mybir.dt.float32, tag="tr")
                            nc.tensor.transpose(
                                trp[:], fn_tiles[c][:, i * D : (i + 1) * D], eye_tile[:]
                            )
                            nc.vector.tensor_copy(
                                ftl_tile[:, c * 128 : (c + 1) * 128], trp[:]
                            )

                    w_tile = w_pool.tile([128, s * D], mybir.dt.float32, tag="w")
                    nc.gpsimd.dma_start(w_tile[:], w_t[:, p0 * D : (p0 + s) * D])

                    if wide:
                        # One output DMA per pair-window covering all 4 batch
                        # chunks (bigger transfers, better HBM write efficiency).
                        out3 = out.reshape([NCHUNK, 128, PD])
                        for w0 in range(0, s, wide):
                            nw = min(wide, s - w0)
                            stg_tile = stg_pool.tile(
                                [128, NCHUNK * wide * D], mybir.dt.float32, tag="stg"
                            )
                            for c in range(NCHUNK):
                                for q in range(w0, w0 + nw, GROUP):
                                    ng = min(GROUP, w0 + nw - q)
                                    ps = ps_pool.tile(
                                        [128, GROUP * D], mybir.dt.float32, tag="ps"
                                    )
                                    nc.tensor.matmul(
                                        ps[:, : ng * D],
                                        ftl_tile[:, c * 128 : (c + 1) * 128],
                                        w_tile[:, q * D : (q + ng) * D],
                                        start=True,
                                        stop=True,
                                    )
                                    j0 = i + 1 + q
                                    off = (c * nw + (q - w0)) * D
                                    nc.vector.tensor_mul(
                                        stg_tile[:, off : off + ng * D],
                                        ps[:, : ng * D],
                                        fn_tiles[c][:, j0 * D : (j0 + ng) * D],
                                    )
                            out_eng = nc.sync if (i + w0) % 2 == 0 else nc.scalar
                            out_eng.dma_start(
                                out3[:, :, (p0 + w0) * D : (p0 + w0 + nw) * D]
                                .transpose((1, 0, 2)),
                                stg_tile[:, : NCHUNK * nw * D],
                            )
                        p0 += s
                        continue
                    for c in range(NCHUNK):
                        stg_tile = stg_pool.tile([128, s * D], mybir.dt.float32, tag="stg")
                        if ablate == "nocompute":
                            nc.vector.tensor_scalar_mul(
                                stg_tile[:, 0:4], stg_tile[:, 0:4], 0.0
                            )
                        for q in range(0, s, GROUP) if ablate != "nocompute" else []:
                            ng = min(GROUP, s - q)
                            ps = ps_pool.tile([128, GROUP * D], mybir.dt.float32, tag="ps")
                            lhsT = ftl_tile[:, c * 128 : (c + 1) * 128]  # [K=d, M=b]
                            rhs = w_tile[:, q * D : (q + ng) * D]  # [K=d, N=pairs*e]
                            if mm_dt is not None:
                                lhsT = lhsT.bitcast(mm_dt)
                                rhs = rhs.bitcast(mm_dt)
                            nc.tensor.matmul(ps[:, : ng * D], lhsT, rhs, start=True, stop=True)
                            j0 = i + 1 + q
                            nc.vector.tensor_mul(
                                stg_tile[:, q * D : (q + ng) * D],
                                ps[:, : ng * D],
                                fn_tiles[c][:, j0 * D : (j0 + ng) * D],
                            )
                        if ablate != "noout":
                            rings = [nc.sync, nc.scalar, nc.gpsimd][:out_rings]
                            out_eng = rings[(i * NCHUNK + c) % len(rings)]
                            out_eng.dma_start(
                                out[c * 128 : (c + 1) * 128, p0 * D : (p0 + s) * D],
                                stg_tile[:],
                            )
                    p0 += s

    nc.compile()
    return nc


def _input_names(nc):
    names = set()
    for alloc in nc.m.functions[0].allocations:
        if isinstance(alloc, mybir.MemoryLocationSet) and alloc.kind == "ExternalInput":
            names.add(alloc.memorylocations[0].name)
    return names


def kernel(feature_emb, W):
    global last_results
    femb = np.ascontiguousarray(feature_emb, dtype=np.float32)
    Wc = np.asarray(W, dtype=np.float32)
    assert femb.shape == (B, F, D) and Wc.shape == (P, D, D)

    if _cache.get("mode") != MODE:
        _cache["nc"] = _build(mode=MODE)
        _cache["mode"] = MODE
    nc = _cache["nc"]

    # w_t[d, p*D + e] = W[p, e, d]
    w_t = np.ascontiguousarray(Wc.transpose(2, 0, 1)).reshape(D, PD)
    eye = np.eye(D, dtype=np.float32)
    ft_all = femb.transpose(1, 2, 0)  # [F, D, B] view
    in_maps = []
    for co in range(NCORES):
        sl = slice(co * BSH, (co + 1) * BSH)
        in_maps.append(
            {
                "femb_n": femb[sl].reshape(BSH, FD),
                "femb_t": np.ascontiguousarray(ft_all[:, :, sl]).reshape(FD, BSH),
                "w_t": w_t,
                "eye": eye,
            }
        )
    in_maps = [
        {k: v for k, v in m.items() if k in _input_names(nc)} for m in in_maps
    ]

    res = run_bass_kernel_spmd(nc, in_maps, list(range(NCORES)), trace=TRACE)
    last_results = res

    out = np.empty((B, P, D), dtype=np.float32)
    for co in range(NCORES):
        o = res.results[co]["out"]
        if MODE == "eb":
            out[co * BSH : (co + 1) * BSH] = o.reshape(P, D, BSH).transpose(2, 0, 1)
        else:
            out[co * BSH : (co + 1) * BSH] = o.reshape(BSH, P, D)
    return out


# ---------------------------------------------------------------------------
# Timing support (used by test.py; not needed for grading correctness).
# The local axon build has no NTFF profile hook, so HW time is measured as the
# marginal wall-clock of an in-NEFF repeat loop with device-resident inputs:
# t(niter=N) - t(niter=1) cancels all host/tunnel/launch constants.
# ---------------------------------------------------------------------------


def _make_runner(nc, n_cores=NCORES):
    import jax
    import jax.numpy as jnp
    from jax.sharding import Mesh, NamedSharding, PartitionSpec
    from jax.experimental.shard_map import shard_map

    from concourse import bass2jax

    bass2jax.install_neuronx_cc_hook()
    partition_name = nc.partition_id_tensor.name if nc.partition_id_tensor else None
    in_names, out_names, out_avals = [], [], []
    for alloc in nc.m.functions[0].allocations:
        if not isinstance(alloc, mybir.MemoryLocationSet):
            continue
        name = alloc.memorylocations[0].name
        if alloc.kind == "ExternalInput":
            if name != partition_name:
                in_names.append(name)
        elif alloc.kind == "ExternalOutput":
            out_names.append(name)
            out_avals.append(
                jax.core.ShapedArray(tuple(alloc.tensor_shape), mybir.dt.np(alloc.dtype))
            )
    n_params, n_outs = len(in_names), len(out_names)
    all_names = in_names + out_names + ([partition_name] if partition_name else [])

    def _body(*args):
        operands = list(args)
        if partition_name is not None:
            operands.append(bass2jax.partition_id_tensor())
        return tuple(
            bass2jax._bass_exec_p.bind(
                *operands,
                out_avals=tuple(out_avals),
                in_names=tuple(all_names),
                out_names=tuple(out_names),
                lowering_input_output_aliases=(),
                sim_require_finite=True,
                sim_require_nnan=True,
                nc=nc,
            )
        )

    mesh = Mesh(np.asarray(jax.devices()[:n_cores]), ("core",))
    spec = PartitionSpec("core")
    sharded = jax.jit(
        shard_map(
            _body,
            mesh=mesh,
            in_specs=(spec,) * (n_params + n_outs),
            out_specs=(spec,) * n_outs,
            check_rep=False,
        ),
        donate_argnums=tuple(range(n_params, n_params + n_outs)),
        keep_unused=True,
    )
    sharding = NamedSharding(mesh, spec)
    zeros_fn = jax.jit(
        lambda: tuple(
            jnp.zeros((n_cores * a.shape[0], *a.shape[1:]), a.dtype) for a in out_avals
        ),
        out_shardings=(sharding,) * n_outs,
    )
    return sharded, zeros_fn, in_names, sharding


def _bench_once(niter, in_maps, reps=4):
    import time

    import jax

    nc = _build(niter=niter, mode=MODE)
    sharded, zeros_fn, in_names, sharding = _make_runner(nc)
    dev_in = [
        jax.device_put(np.concatenate([m[n] for m in in_maps], axis=0), sharding)
        for n in in_names
    ]
    for a in dev_in:
        a.block_until_ready()
    times = []
    for _ in range(reps):
        zeros = zeros_fn()
        for z in zeros:
            z.block_until_ready()
        t0 = time.time()
        outs = sharded(*dev_in, *zeros)
        for o in outs:
            o.block_until_ready()
        times.append(time.time() - t0)
    return min(times)


def measure_hw_time_ns(feature_emb, W, niter=101, reps=5):
    """Marginal per-iteration HW time of the kernel NEFF, in ns."""
    femb = np.ascontiguousarray(feature_emb, dtype=np.float32)
    Wc = np.asarray(W, dtype=np.float32)
    w_t = np.ascontiguousarray(Wc.transpose(2, 0, 1)).reshape(D, PD)
    eye = np.eye(D, dtype=np.float32)
    ft_all = femb.transpose(1, 2, 0)
    in_maps = []
    for co in range(NCORES):
        sl = slice(co * BSH, (co + 1) * BSH)
        in_maps.append(
            {
                "femb_n": femb[sl].reshape(BSH, FD),
                "femb_t": np.ascontiguousarray(ft_all[:, :, sl]).reshape(FD, BSH),
                "w_t": w_t,
                "eye": eye,
            }
        )
    t1 = _bench_once(1, in_maps, reps)
    tn = _bench_once(niter, in_maps, reps)
    return (tn - t1) / (niter - 1) * 1e9, t1, tn



# revision 5
# speedup vs baseline: 1.6557x; 1.6557x over previous
"""Bilinear interaction layer (pairwise per-field Linear + gate) on 8 trn2 cores.

out[b, p, :] = (femb[b, i_p] @ W[p].T) * femb[b, j_p]   for the P=C(F,2) field
pairs (i_p, j_p) in itertools.combinations order.  B=4096, F=30, D=128, P=435.

Sharding: data-parallel over batch (4096 -> 512 per core), W replicated.

Per core, pairs are processed in "i-blocks" (the (29-i) pairs sharing first
field i, consecutive in p).  For each i-block and each 128-row batch chunk,
TensorE runs fp32 matmuls with the v_i chunk [d=128, b=128] stationary and up
to 4 pairs' transposed weights [d=128, 4*128] moving (N=512, the fp32 moving
limit), producing PSUM [b=128, 4*128] directly in the natural output layout.
VectorE applies the v_j gate straight out of PSUM into an SBUF staging tile
(fused PSUM-read + multiply + SBUF-write), and the staging tile is DMA'd out
with >= 512B-contiguous rows.

DMA engine assignment: all input loads go through SWDGE (GpSimd) so they can
never queue behind backpressured output stores; output stores alternate
between the two HWDGE rings (SP and ACT).  Per core traffic: 44 MB in (W
28.5 + two embedding layouts 15.3) + 114 MB out.  Measured on HW (marginal
time of an in-NEFF repeat loop): ~0.50 ms/call -- simultaneously at the PE
fp32 limit (480 self-loading fp32 matmuls) and the HBM limit.  Output is
bit-identical to an fp32 jax reference on-device and ~2e-7 Frobenius relative
error vs CPU BLAS.
"""

import os
import sys

import numpy as np

for _p in ("/opt/trn_rl_repo", "/root/.axon_site/_ro/trn_rl_repo"):
    if os.path.isdir(_p) and _p not in sys.path:
        sys.path.append(_p)

import concourse.bacc as bacc
import concourse.tile as tile
from concourse import mybir
from concourse.bass_utils import run_bass_kernel_spmd

B, F, D = 4096, 30, 128
P = F * (F - 1) // 2  # 435
NCORES = 8
BSH = B // NCORES  # 512 batches per core
NCHUNK = BSH // 128  # 4 batch chunks of 128
GROUP = 4  # pairs per matmul -> moving dim 512 (fp32 max)
FD = F * D  # 3840
PD = P * D  # 55680

MODE = "bf16"  # "bf16" | "load" (natural-layout output) | "eb" ([e,b] compute layout)
ACT_EVERY = 3  # bf16 mode: of every ACT_EVERY matmul groups, ACT_EVERY-1 gate via
#               ACT-copy + packed DVE TT; 0 = all gates direct on DVE
TRACE = False
last_results = None  # BassKernelResults of the most recent kernel() call

_cache = {}

import ml_dtypes

BF16 = ml_dtypes.bfloat16


def _build_bf16(niter=1, act_every=None, ps_bufs=6, stg_bufs=4, w_bufs=3, scr_bufs=4):
    """bf16 variant: all DRAM tensors bf16, matmul in bf16 (4x PE rate vs fp32),
    staged output bf16 (half the store traffic).  PSUM stays fp32 (TRN2 rule).
    Gate multiply is split between two paths to keep every engine under the
    DMA roofline:
      - direct:  DVE tensor_mul(stg_bf16, psum_fp32, femb_bf16)  -> 1x mode
      - offload: ACT copy psum_fp32 -> scr_bf16, then DVE tensor_mul on two
                 packed bf16 operands -> 2x_1P mode (2 elem/cycle/lane)
    """
    if act_every is None:
        act_every = ACT_EVERY
    nc = bacc.Bacc("TRN2", target_bir_lowering=False, debug=False, num_devices=NCORES)
    femb_n = nc.declare_dram_parameter("femb_n", [BSH, FD], mybir.dt.bfloat16, isOutput=False)
    femb_t = nc.declare_dram_parameter("femb_t", [FD, BSH], mybir.dt.bfloat16, isOutput=False)
    w_t = nc.declare_dram_parameter("w_t", [D, PD], mybir.dt.bfloat16, isOutput=False)
    out = nc.declare_dram_parameter("out", [BSH, PD], mybir.dt.bfloat16, isOutput=True)

    import contextlib

    with tile.TileContext(nc) as tc:
        with (
            tc.tile_pool(name="fn", bufs=1) as fn_pool,
            tc.tile_pool(name="ftl", bufs=3) as ftl_pool,
            tc.tile_pool(name="w", bufs=w_bufs) as w_pool,
            tc.tile_pool(name="stg", bufs=stg_bufs) as stg_pool,
            tc.tile_pool(name="scr", bufs=scr_bufs) as scr_pool,
            tc.tile_pool(name="ps", bufs=ps_bufs, space="PSUM") as ps_pool,
            tc.For_i(
                0,
                niter,
                1,
                hint_engines=(
                    mybir.EngineType.PE,
                    mybir.EngineType.DVE,
                    mybir.EngineType.Activation,
                    mybir.EngineType.SP,
                ),
            )
            if niter > 1
            else contextlib.nullcontext(),
        ):
            fn_tiles = []
            for c in range(NCHUNK):
                fnt = fn_pool.tile([128, FD], mybir.dt.bfloat16, tag=f"fn{c}")
                nc.gpsimd.dma_start(fnt[:], femb_n[c * 128 : (c + 1) * 128, :])
                fn_tiles.append(fnt)

            grp = 0  # matmul-group counter for the gate-path round robin
            p0 = 0
            for i in range(F - 1):
                s = F - 1 - i  # pairs in this i-block: (i, i+1) .. (i, F-1)
                ftl_tile = ftl_pool.tile([128, BSH], mybir.dt.bfloat16, tag="ftl")
                nc.gpsimd.dma_start(ftl_tile[:], femb_t[i * D : (i + 1) * D, :])

                w_tile = w_pool.tile([128, s * D], mybir.dt.bfloat16, tag="w")
                nc.gpsimd.dma_start(w_tile[:], w_t[:, p0 * D : (p0 + s) * D])

                for c in range(NCHUNK):
                    stg_tile = stg_pool.tile([128, s * D], mybir.dt.bfloat16, tag="stg")
                    for q in range(0, s, GROUP):
                        ng = min(GROUP, s - q)
                        ps = ps_pool.tile([128, GROUP * D], mybir.dt.float32, tag="ps")
                        nc.tensor.matmul(
                            ps[:, : ng * D],
                            ftl_tile[:, c * 128 : (c + 1) * 128],  # [K=d, M=b]
                            w_tile[:, q * D : (q + ng) * D],  # [K=d, N=pairs*e]
                            start=True,
                            stop=True,
                        )
                        j0 = i + 1 + q
                        if act_every and grp % act_every != 0:
                            scr = scr_pool.tile(
                                [128, GROUP * D], mybir.dt.bfloat16, tag="scr"
                            )
                            nc.scalar.copy(scr[:, : ng * D], ps[:, : ng * D])
                            nc.vector.tensor_mul(
                                stg_tile[:, q * D : (q + ng) * D],
                                scr[:, : ng * D],
                                fn_tiles[c][:, j0 * D : (j0 + ng) * D],
                            )
                        else:
                            nc.vector.tensor_mul(
                                stg_tile[:, q * D : (q + ng) * D],
                                ps[:, : ng * D],
                                fn_tiles[c][:, j0 * D : (j0 + ng) * D],
                            )
                        grp += 1
                    out_eng = nc.sync if (i * NCHUNK + c) % 2 == 0 else nc.scalar
                    out_eng.dma_start(
                        out[c * 128 : (c + 1) * 128, p0 * D : (p0 + s) * D],
                        stg_tile[:],
                    )
                p0 += s

    nc.compile()
    return nc


def _build(niter=1, mode="load", ftl_bufs=3, mm_dt=None, ps_bufs=None, ablate=None, stg_bufs=4, w_bufs=3, wide=0, out_rings=2):
    nc = bacc.Bacc("TRN2", target_bir_lowering=False, debug=False, num_devices=NCORES)
    if mode != "eb":
        femb_n = nc.declare_dram_parameter("femb_n", [BSH, FD], mybir.dt.float32, isOutput=False)
    if mode in ("load", "eb"):
        femb_t = nc.declare_dram_parameter("femb_t", [FD, BSH], mybir.dt.float32, isOutput=False)
    w_t = nc.declare_dram_parameter("w_t", [D, PD], mybir.dt.float32, isOutput=False)
    if mode != "load":
        eye = nc.declare_dram_parameter("eye", [D, D], mybir.dt.float32, isOutput=False)
    if mode == "eb":
        out = nc.declare_dram_parameter("out", [PD, BSH], mybir.dt.float32, isOutput=True)
    else:
        out = nc.declare_dram_parameter("out", [BSH, PD], mybir.dt.float32, isOutput=True)

    import contextlib

    with tile.TileContext(nc) as tc:
        with (
            tc.tile_pool(name="eye", bufs=1) as eye_pool,
            tc.tile_pool(name="fn", bufs=1) as fn_pool,
            tc.tile_pool(name="ftl", bufs=ftl_bufs) as ftl_pool,
            tc.tile_pool(name="w", bufs=w_bufs) as w_pool,
            tc.tile_pool(name="stg", bufs=stg_bufs) as stg_pool,
            tc.tile_pool(name="ps", bufs=ps_bufs or 6, space="PSUM") as ps_pool,
            tc.tile_pool(name="tr", bufs=2, space="PSUM") as tr_pool,
            tc.For_i(
                0,
                niter,
                1,
                hint_engines=(
                    mybir.EngineType.PE,
                    mybir.EngineType.DVE,
                    mybir.EngineType.Activation,
                    mybir.EngineType.SP,
                ),
            )
            if niter > 1
            else contextlib.nullcontext(),
        ):
            if mode == "eb":
                # [e, b] layout: W stationary, activations moving. All of
                # femb_t stays resident (60 KB/partition); output tensor is
                # [P*D, BSH] so every store is a fully-sequential DRAM block.
                # Host un-transposes the result.
                WIN = 8
                femb_all = fn_pool.tile([128, F * BSH], mybir.dt.float32, tag="fa")
                for f in range(F):
                    nc.gpsimd.dma_start(
                        femb_all[:, f * BSH : (f + 1) * BSH],
                        femb_t[f * D : (f + 1) * D, :],
                    )
                out3 = out.reshape([P, D, BSH])
                p0 = 0
                for i in range(F - 1):
                    s = F - 1 - i
                    w_tile = w_pool.tile([128, s * D], mybir.dt.float32, tag="w")
                    nc.gpsimd.dma_start(w_tile[:], w_t[:, p0 * D : (p0 + s) * D])
                    for w0 in range(0, s, WIN):
                        nw = min(WIN, s - w0)
                        stg_tile = stg_pool.tile(
                            [128, WIN, BSH], mybir.dt.float32, tag="stg"
                        )
                        for k in range(nw):
                            pr = w0 + k
                            j = i + 1 + pr
                            ps = ps_pool.tile([128, BSH], mybir.dt.float32, tag="ps")
                            nc.tensor.matmul(
                                ps[:],
                                w_tile[:, pr * D : (pr + 1) * D],  # [K=d, M=e]
                                femb_all[:, i * BSH : (i + 1) * BSH],  # [K=d, N=b]
                                start=True,
                                stop=True,
                            )
                            nc.vector.tensor_mul(
                                stg_tile[:, k, :],
                                ps[:],
                                femb_all[:, j * BSH : (j + 1) * BSH],
                            )
                        rings = [nc.sync, nc.scalar][:out_rings]
                        out_eng = rings[(p0 + w0) % len(rings)]
                        out_eng.dma_start(
                            out3[p0 + w0 : p0 + w0 + nw, :, :].transpose((1, 0, 2)),
                            stg_tile[:, :nw, :],
                        )
                    p0 += s
            else:
                if mode != "load":
                    eye_tile = eye_pool.tile([D, D], mybir.dt.float32)
                    nc.gpsimd.dma_start(eye_tile[:], eye[:])
                # whole femb shard, natural layout: partition=b (within chunk),
                # free=(field, emb); one tile per batch chunk so consumers only
                # wait on the chunk they need.
                fn_tiles = []
                for c in range(NCHUNK):
                    fnt = fn_pool.tile([128, FD], mybir.dt.float32, tag=f"fn{c}")
                    nc.gpsimd.dma_start(fnt[:], femb_n[c * 128 : (c + 1) * 128, :])
                    fn_tiles.append(fnt)

                p0 = 0
                for i in range(F - 1):
                    s = F - 1 - i  # pairs in this i-block: (i, i+1) .. (i, F-1)
                    # Build v_i in [d, b] layout on-chip: PE transpose-mode
                    # (exact data movement) + ScalarE copy out of PSUM.
                    ftl_tile = ftl_pool.tile([128, BSH], mybir.dt.float32, tag="ftl")
                    if mode == "load":
                        nc.gpsimd.dma_start(ftl_tile[:], femb_t[i * D : (i + 1) * D, :])
                    else:
                        for c in range(NCHUNK):
                            trp = tr_pool.tile([128, 128], mybir.dt.float32, tag="tr")
                            nc.tensor.transpose(
                                trp[:], fn_tiles[c][:, i * D : (i + 1) * D], eye_tile[:]
                            )
                            nc.vector.tensor_copy(
                                ftl_tile[:, c * 128 : (c + 1) * 128], trp[:]
                            )

                    w_tile = w_pool.tile([128, s * D], mybir.dt.float32, tag="w")
                    nc.gpsimd.dma_start(w_tile[:], w_t[:, p0 * D : (p0 + s) * D])

                    if wide:
                        # One output DMA per pair-window covering all 4 batch
                        # chunks (bigger transfers, better HBM write efficiency).
                        out3 = out.reshape([NCHUNK, 128, PD])
                        for w0 in range(0, s, wide):
                            nw = min(wide, s - w0)
                            stg_tile = stg_pool.tile(
                                [128, NCHUNK * wide * D], mybir.dt.float32, tag="stg"
                            )
                            for c in range(NCHUNK):
                                for q in range(w0, w0 + nw, GROUP):
                                    ng = min(GROUP, w0 + nw - q)
                                    ps = ps_pool.tile(
                                        [128, GROUP * D], mybir.dt.float32, tag="ps"
                                    )
                                    nc.tensor.matmul(
                                        ps[:, : ng * D],
                                        ftl_tile[:, c * 128 : (c + 1) * 128],
                                        w_tile[:, q * D : (q + ng) * D],
                                        start=True,
                                        stop=True,
                                    )
                                    j0 = i + 1 + q
                                    off = (c * nw + (q - w0)) * D
                                    nc.vector.tensor_mul(
                                        stg_tile[:, off : off + ng * D],
                                        ps[:, : ng * D],
                                        fn_tiles[c][:, j0 * D : (j0 + ng) * D],
                                    )
                            out_eng = nc.sync if (i + w0) % 2 == 0 else nc.scalar
                            out_eng.dma_start(
                                out3[:, :, (p0 + w0) * D : (p0 + w0 + nw) * D]
                                .transpose((1, 0, 2)),
                                stg_tile[:, : NCHUNK * nw * D],
                            )
                        p0 += s
                        continue
                    for c in range(NCHUNK):
                        stg_tile = stg_pool.tile([128, s * D], mybir.dt.float32, tag="stg")
                        if ablate == "nocompute":
                            nc.vector.tensor_scalar_mul(
                                stg_tile[:, 0:4], stg_tile[:, 0:4], 0.0
                            )
                        for q in range(0, s, GROUP) if ablate != "nocompute" else []:
                            ng = min(GROUP, s - q)
                            ps = ps_pool.tile([128, GROUP * D], mybir.dt.float32, tag="ps")
                            lhsT = ftl_tile[:, c * 128 : (c + 1) * 128]  # [K=d, M=b]
                            rhs = w_tile[:, q * D : (q + ng) * D]  # [K=d, N=pairs*e]
                            if mm_dt is not None:
                                lhsT = lhsT.bitcast(mm_dt)
                                rhs = rhs.bitcast(mm_dt)
                            nc.tensor.matmul(ps[:, : ng * D], lhsT, rhs, start=True, stop=True)
                            j0 = i + 1 + q
                            nc.vector.tensor_mul(
                                stg_tile[:, q * D : (q + ng) * D],
                                ps[:, : ng * D],
                                fn_tiles[c][:, j0 * D : (j0 + ng) * D],
                            )
                        if ablate != "noout":
                            rings = [nc.sync, nc.scalar, nc.gpsimd][:out_rings]
                            out_eng = rings[(i * NCHUNK + c) % len(rings)]
                            out_eng.dma_start(
                                out[c * 128 : (c + 1) * 128, p0 * D : (p0 + s) * D],
                                stg_tile[:],
                            )
                    p0 += s

    nc.compile()
    return nc


def _input_names(nc):
    names = set()
    for alloc in nc.m.functions[0].allocations:
        if isinstance(alloc, mybir.MemoryLocationSet) and alloc.kind == "ExternalInput":
            names.add(alloc.memorylocations[0].name)
    return names


def _prep_in_maps(femb, Wc, names):
    """Per-core input maps (full-precision host arrays -> device layouts)."""
    # w_t[d, p*D + e] = W[p, e, d]
    w_t = np.ascontiguousarray(Wc.transpose(2, 0, 1)).reshape(D, PD)
    eye = np.eye(D, dtype=np.float32)
    if MODE == "bf16":
        femb = femb.astype(BF16)
        w_t = w_t.astype(BF16)
    ft_all = femb.transpose(1, 2, 0)  # [F, D, B] view
    in_maps = []
    for co in range(NCORES):
        sl = slice(co * BSH, (co + 1) * BSH)
        m = {
            "femb_n": femb[sl].reshape(BSH, FD),
            "femb_t": np.ascontiguousarray(ft_all[:, :, sl]).reshape(FD, BSH),
            "w_t": w_t,
            "eye": eye,
        }
        in_maps.append({k: v for k, v in m.items() if k in names})
    return in_maps


def _get_nc():
    if _cache.get("mode") != MODE:
        _cache["nc"] = _build_bf16() if MODE == "bf16" else _build(mode=MODE)
        _cache["mode"] = MODE
    return _cache["nc"]


def kernel(feature_emb, W):
    global last_results
    femb = np.ascontiguousarray(feature_emb, dtype=np.float32)
    Wc = np.asarray(W, dtype=np.float32)
    assert femb.shape == (B, F, D) and Wc.shape == (P, D, D)

    nc = _get_nc()
    in_maps = _prep_in_maps(femb, Wc, _input_names(nc))

    res = run_bass_kernel_spmd(nc, in_maps, list(range(NCORES)), trace=TRACE)
    last_results = res

    out = np.empty((B, P, D), dtype=np.float32)
    for co in range(NCORES):
        o = res.results[co]["out"]
        if MODE == "eb":
            out[co * BSH : (co + 1) * BSH] = o.reshape(P, D, BSH).transpose(2, 0, 1)
        else:
            out[co * BSH : (co + 1) * BSH] = np.asarray(
                o, dtype=np.float32
            ).reshape(BSH, P, D)
    return out


# ---------------------------------------------------------------------------
# Timing support (used by test.py; not needed for grading correctness).
# The local axon build has no NTFF profile hook, so HW time is measured as the
# marginal wall-clock of an in-NEFF repeat loop with device-resident inputs:
# t(niter=N) - t(niter=1) cancels all host/tunnel/launch constants.
# ---------------------------------------------------------------------------


def _make_runner(nc, n_cores=NCORES):
    import jax
    import jax.numpy as jnp
    from jax.sharding import Mesh, NamedSharding, PartitionSpec
    from jax.experimental.shard_map import shard_map

    from concourse import bass2jax

    bass2jax.install_neuronx_cc_hook()
    partition_name = nc.partition_id_tensor.name if nc.partition_id_tensor else None
    in_names, out_names, out_avals = [], [], []
    for alloc in nc.m.functions[0].allocations:
        if not isinstance(alloc, mybir.MemoryLocationSet):
            continue
        name = alloc.memorylocations[0].name
        if alloc.kind == "ExternalInput":
            if name != partition_name:
                in_names.append(name)
        elif alloc.kind == "ExternalOutput":
            out_names.append(name)
            out_avals.append(
                jax.core.ShapedArray(tuple(alloc.tensor_shape), mybir.dt.np(alloc.dtype))
            )
    n_params, n_outs = len(in_names), len(out_names)
    all_names = in_names + out_names + ([partition_name] if partition_name else [])

    def _body(*args):
        operands = list(args)
        if partition_name is not None:
            operands.append(bass2jax.partition_id_tensor())
        return tuple(
            bass2jax._bass_exec_p.bind(
                *operands,
                out_avals=tuple(out_avals),
                in_names=tuple(all_names),
                out_names=tuple(out_names),
                lowering_input_output_aliases=(),
                sim_require_finite=True,
                sim_require_nnan=True,
                nc=nc,
            )
        )

    mesh = Mesh(np.asarray(jax.devices()[:n_cores]), ("core",))
    spec = PartitionSpec("core")
    sharded = jax.jit(
        shard_map(
            _body,
            mesh=mesh,
            in_specs=(spec,) * (n_params + n_outs),
            out_specs=(spec,) * n_outs,
            check_rep=False,
        ),
        donate_argnums=tuple(range(n_params, n_params + n_outs)),
        keep_unused=True,
    )
    sharding = NamedSharding(mesh, spec)
    zeros_fn = jax.jit(
        lambda: tuple(
            jnp.zeros((n_cores * a.shape[0], *a.shape[1:]), a.dtype) for a in out_avals
        ),
        out_shardings=(sharding,) * n_outs,
    )
    return sharded, zeros_fn, in_names, sharding


def _bench_once(niter, in_maps, reps=4):
    import time

    import jax

    nc = _build_bf16(niter=niter) if MODE == "bf16" else _build(niter=niter, mode=MODE)
    sharded, zeros_fn, in_names, sharding = _make_runner(nc)
    dev_in = [
        jax.device_put(np.concatenate([m[n] for m in in_maps], axis=0), sharding)
        for n in in_names
    ]
    for a in dev_in:
        a.block_until_ready()
    times = []
    for _ in range(reps):
        zeros = zeros_fn()
        for z in zeros:
            z.block_until_ready()
        t0 = time.time()
        outs = sharded(*dev_in, *zeros)
        for o in outs:
            o.block_until_ready()
        times.append(time.time() - t0)
    return min(times)


def measure_hw_time_ns(feature_emb, W, niter=101, reps=5):
    """Marginal per-iteration HW time of the kernel NEFF, in ns."""
    femb = np.ascontiguousarray(feature_emb, dtype=np.float32)
    Wc = np.asarray(W, dtype=np.float32)
    names = _input_names(_get_nc())
    in_maps = _prep_in_maps(femb, Wc, names)
    t1 = _bench_once(1, in_maps, reps)
    tn = _bench_once(niter, in_maps, reps)
    return (tn - t1) / (niter - 1) * 1e9, t1, tn



# revision 35
# speedup vs baseline: 2.1194x; 1.2801x over previous
"""Bilinear interaction layer (pairwise per-field Linear + gate) on 8 trn2 cores.

out[b, p, :] = (femb[b, i_p] @ W[p].T) * femb[b, j_p]   for the P=C(F,2) field
pairs (i_p, j_p) in itertools.combinations order.  B=4096, F=30, D=128, P=435.

Sharding: data-parallel over batch (4096 -> 512 per core), W replicated.

Per core, pairs are processed in "i-blocks" (the (29-i) pairs sharing first
field i, consecutive in p).  For each i-block and each 128-row batch chunk,
TensorE runs fp32 matmuls with the v_i chunk [d=128, b=128] stationary and up
to 4 pairs' transposed weights [d=128, 4*128] moving (N=512, the fp32 moving
limit), producing PSUM [b=128, 4*128] directly in the natural output layout.
VectorE applies the v_j gate straight out of PSUM into an SBUF staging tile
(fused PSUM-read + multiply + SBUF-write), and the staging tile is DMA'd out
with >= 512B-contiguous rows.

DMA engine assignment: all input loads go through SWDGE (GpSimd) so they can
never queue behind backpressured output stores; output stores alternate
between the two HWDGE rings (SP and ACT).  Per core traffic: 44 MB in (W
28.5 + two embedding layouts 15.3) + 114 MB out.  Measured on HW (marginal
time of an in-NEFF repeat loop): ~0.50 ms/call -- simultaneously at the PE
fp32 limit (480 self-loading fp32 matmuls) and the HBM limit.  Output is
bit-identical to an fp32 jax reference on-device and ~2e-7 Frobenius relative
error vs CPU BLAS.
"""

import os
import sys

import numpy as np

for _p in ("/opt/trn_rl_repo", "/root/.axon_site/_ro/trn_rl_repo"):
    if os.path.isdir(_p) and _p not in sys.path:
        sys.path.append(_p)

import concourse.bacc as bacc
import concourse.tile as tile
from concourse import mybir
from concourse.bass_utils import run_bass_kernel_spmd

B, F, D = 4096, 30, 128
P = F * (F - 1) // 2  # 435
NCORES = 8
BSH = B // NCORES  # 512 batches per core
NCHUNK = BSH // 128  # 4 batch chunks of 128
GROUP = 4  # pairs per matmul -> moving dim 512 (fp32 max)
FD = F * D  # 3840
PD = P * D  # 55680

MODE = "v3"  # "v3" | "v2" | "bf16" | "load" | "eb"
ACT_EVERY = 3  # bf16 mode: of every ACT_EVERY matmul groups, ACT_EVERY-1 gate via
#               ACT-copy + packed DVE TT; 0 = all gates direct on DVE
TRACE = False
last_results = None  # BassKernelResults of the most recent kernel() call

_cache = {}

import ml_dtypes

BF16 = ml_dtypes.bfloat16


def _build_bf16(
    niter=1,
    act_every=None,
    ps_bufs=6,
    stg_bufs=4,
    w_bufs=3,
    scr_bufs=4,
    ablate=None,  # None | "noout" | "loadsonly" | "storesonly" | "nogate"
):
    """bf16 variant: all DRAM tensors bf16, matmul in bf16 (4x PE rate vs fp32),
    staged output bf16 (half the store traffic).  PSUM stays fp32 (TRN2 rule).
    Gate multiply is split between two paths to keep every engine under the
    DMA roofline:
      - direct:  DVE tensor_mul(stg_bf16, psum_fp32, femb_bf16)  -> 1x mode
      - offload: ACT copy psum_fp32 -> scr_bf16, then DVE tensor_mul on two
                 packed bf16 operands -> 2x_1P mode (2 elem/cycle/lane)
    """
    if act_every is None:
        act_every = ACT_EVERY
    nc = bacc.Bacc("TRN2", target_bir_lowering=False, debug=False, num_devices=NCORES)
    femb_n = nc.declare_dram_parameter("femb_n", [BSH, FD], mybir.dt.bfloat16, isOutput=False)
    femb_t = nc.declare_dram_parameter("femb_t", [FD, BSH], mybir.dt.bfloat16, isOutput=False)
    w_t = nc.declare_dram_parameter("w_t", [D, PD], mybir.dt.bfloat16, isOutput=False)
    out = nc.declare_dram_parameter("out", [BSH, PD], mybir.dt.bfloat16, isOutput=True)

    import contextlib

    with tile.TileContext(nc) as tc:
        with (
            tc.tile_pool(name="fn", bufs=1) as fn_pool,
            tc.tile_pool(name="ftl", bufs=3) as ftl_pool,
            tc.tile_pool(name="w", bufs=w_bufs) as w_pool,
            tc.tile_pool(name="stg", bufs=stg_bufs) as stg_pool,
            tc.tile_pool(name="scr", bufs=scr_bufs) as scr_pool,
            tc.tile_pool(name="ps", bufs=ps_bufs, space="PSUM") as ps_pool,
            tc.For_i(
                0,
                niter,
                1,
                hint_engines=(
                    mybir.EngineType.PE,
                    mybir.EngineType.DVE,
                    mybir.EngineType.Activation,
                    mybir.EngineType.SP,
                ),
            )
            if niter > 1
            else contextlib.nullcontext(),
        ):
            do_loads = ablate not in ("storesonly",)
            do_compute = ablate not in ("loadsonly", "storesonly")
            do_gate = do_compute and ablate != "nogate"
            do_stores = ablate not in ("noout", "loadsonly")

            fn_tiles = []
            for c in range(NCHUNK):
                fnt = fn_pool.tile([128, FD], mybir.dt.bfloat16, tag=f"fn{c}")
                if do_loads:
                    nc.gpsimd.dma_start(fnt[:], femb_n[c * 128 : (c + 1) * 128, :])
                fn_tiles.append(fnt)

            grp = 0  # matmul-group counter for the gate-path round robin
            p0 = 0
            for i in range(F - 1):
                s = F - 1 - i  # pairs in this i-block: (i, i+1) .. (i, F-1)
                ftl_tile = ftl_pool.tile([128, BSH], mybir.dt.bfloat16, tag="ftl")
                w_tile = w_pool.tile([128, s * D], mybir.dt.bfloat16, tag="w")
                if do_loads:
                    nc.gpsimd.dma_start(ftl_tile[:], femb_t[i * D : (i + 1) * D, :])
                    nc.gpsimd.dma_start(w_tile[:], w_t[:, p0 * D : (p0 + s) * D])

                for c in range(NCHUNK):
                    stg_tile = stg_pool.tile([128, s * D], mybir.dt.bfloat16, tag="stg")
                    if not do_gate and do_stores:
                        # touch the tile so the store has a producer
                        nc.vector.tensor_scalar_mul(
                            stg_tile[:, 0:4], stg_tile[:, 0:4], 0.0
                        )
                    for q in range(0, s, GROUP) if do_compute else []:
                        ng = min(GROUP, s - q)
                        ps = ps_pool.tile([128, GROUP * D], mybir.dt.float32, tag="ps")
                        nc.tensor.matmul(
                            ps[:, : ng * D],
                            ftl_tile[:, c * 128 : (c + 1) * 128],  # [K=d, M=b]
                            w_tile[:, q * D : (q + ng) * D],  # [K=d, N=pairs*e]
                            start=True,
                            stop=True,
                        )
                        j0 = i + 1 + q
                        if not do_gate:
                            grp += 1
                            continue
                        if act_every and grp % act_every != 0:
                            scr = scr_pool.tile(
                                [128, GROUP * D], mybir.dt.bfloat16, tag="scr"
                            )
                            nc.scalar.copy(scr[:, : ng * D], ps[:, : ng * D])
                            nc.vector.tensor_mul(
                                stg_tile[:, q * D : (q + ng) * D],
                                scr[:, : ng * D],
                                fn_tiles[c][:, j0 * D : (j0 + ng) * D],
                            )
                        else:
                            nc.vector.tensor_mul(
                                stg_tile[:, q * D : (q + ng) * D],
                                ps[:, : ng * D],
                                fn_tiles[c][:, j0 * D : (j0 + ng) * D],
                            )
                        grp += 1
                    if do_stores:
                        out_eng = nc.sync if (i * NCHUNK + c) % 2 == 0 else nc.scalar
                        out_eng.dma_start(
                            out[c * 128 : (c + 1) * 128, p0 * D : (p0 + s) * D],
                            stg_tile[:],
                        )
                p0 += s

    nc.compile()
    return nc


def _build(niter=1, mode="load", ftl_bufs=3, mm_dt=None, ps_bufs=None, ablate=None, stg_bufs=4, w_bufs=3, wide=0, out_rings=2):
    nc = bacc.Bacc("TRN2", target_bir_lowering=False, debug=False, num_devices=NCORES)
    if mode != "eb":
        femb_n = nc.declare_dram_parameter("femb_n", [BSH, FD], mybir.dt.float32, isOutput=False)
    if mode in ("load", "eb"):
        femb_t = nc.declare_dram_parameter("femb_t", [FD, BSH], mybir.dt.float32, isOutput=False)
    w_t = nc.declare_dram_parameter("w_t", [D, PD], mybir.dt.float32, isOutput=False)
    if mode != "load":
        eye = nc.declare_dram_parameter("eye", [D, D], mybir.dt.float32, isOutput=False)
    if mode == "eb":
        out = nc.declare_dram_parameter("out", [PD, BSH], mybir.dt.float32, isOutput=True)
    else:
        out = nc.declare_dram_parameter("out", [BSH, PD], mybir.dt.float32, isOutput=True)

    import contextlib

    with tile.TileContext(nc) as tc:
        with (
            tc.tile_pool(name="eye", bufs=1) as eye_pool,
            tc.tile_pool(name="fn", bufs=1) as fn_pool,
            tc.tile_pool(name="ftl", bufs=ftl_bufs) as ftl_pool,
            tc.tile_pool(name="w", bufs=w_bufs) as w_pool,
            tc.tile_pool(name="stg", bufs=stg_bufs) as stg_pool,
            tc.tile_pool(name="ps", bufs=ps_bufs or 6, space="PSUM") as ps_pool,
            tc.tile_pool(name="tr", bufs=2, space="PSUM") as tr_pool,
            tc.For_i(
                0,
                niter,
                1,
                hint_engines=(
                    mybir.EngineType.PE,
                    mybir.EngineType.DVE,
                    mybir.EngineType.Activation,
                    mybir.EngineType.SP,
                ),
            )
            if niter > 1
            else contextlib.nullcontext(),
        ):
            if mode == "eb":
                # [e, b] layout: W stationary, activations moving. All of
                # femb_t stays resident (60 KB/partition); output tensor is
                # [P*D, BSH] so every store is a fully-sequential DRAM block.
                # Host un-transposes the result.
                WIN = 8
                femb_all = fn_pool.tile([128, F * BSH], mybir.dt.float32, tag="fa")
                for f in range(F):
                    nc.gpsimd.dma_start(
                        femb_all[:, f * BSH : (f + 1) * BSH],
                        femb_t[f * D : (f + 1) * D, :],
                    )
                out3 = out.reshape([P, D, BSH])
                p0 = 0
                for i in range(F - 1):
                    s = F - 1 - i
                    w_tile = w_pool.tile([128, s * D], mybir.dt.float32, tag="w")
                    nc.gpsimd.dma_start(w_tile[:], w_t[:, p0 * D : (p0 + s) * D])
                    for w0 in range(0, s, WIN):
                        nw = min(WIN, s - w0)
                        stg_tile = stg_pool.tile(
                            [128, WIN, BSH], mybir.dt.float32, tag="stg"
                        )
                        for k in range(nw):
                            pr = w0 + k
                            j = i + 1 + pr
                            ps = ps_pool.tile([128, BSH], mybir.dt.float32, tag="ps")
                            nc.tensor.matmul(
                                ps[:],
                                w_tile[:, pr * D : (pr + 1) * D],  # [K=d, M=e]
                                femb_all[:, i * BSH : (i + 1) * BSH],  # [K=d, N=b]
                                start=True,
                                stop=True,
                            )
                            nc.vector.tensor_mul(
                                stg_tile[:, k, :],
                                ps[:],
                                femb_all[:, j * BSH : (j + 1) * BSH],
                            )
                        rings = [nc.sync, nc.scalar][:out_rings]
                        out_eng = rings[(p0 + w0) % len(rings)]
                        out_eng.dma_start(
                            out3[p0 + w0 : p0 + w0 + nw, :, :].transpose((1, 0, 2)),
                            stg_tile[:, :nw, :],
                        )
                    p0 += s
            else:
                if mode != "load":
                    eye_tile = eye_pool.tile([D, D], mybir.dt.float32)
                    nc.gpsimd.dma_start(eye_tile[:], eye[:])
                # whole femb shard, natural layout: partition=b (within chunk),
                # free=(field, emb); one tile per batch chunk so consumers only
                # wait on the chunk they need.
                fn_tiles = []
                for c in range(NCHUNK):
                    fnt = fn_pool.tile([128, FD], mybir.dt.float32, tag=f"fn{c}")
                    nc.gpsimd.dma_start(fnt[:], femb_n[c * 128 : (c + 1) * 128, :])
                    fn_tiles.append(fnt)

                p0 = 0
                for i in range(F - 1):
                    s = F - 1 - i  # pairs in this i-block: (i, i+1) .. (i, F-1)
                    # Build v_i in [d, b] layout on-chip: PE transpose-mode
                    # (exact data movement) + ScalarE copy out of PSUM.
                    ftl_tile = ftl_pool.tile([128, BSH], mybir.dt.float32, tag="ftl")
                    if mode == "load":
                        nc.gpsimd.dma_start(ftl_tile[:], femb_t[i * D : (i + 1) * D, :])
                    else:
                        for c in range(NCHUNK):
                            trp = tr_pool.tile([128, 128], mybir.dt.float32, tag="tr")
                            nc.tensor.transpose(
                                trp[:], fn_tiles[c][:, i * D : (i + 1) * D], eye_tile[:]
                            )
                            nc.vector.tensor_copy(
                                ftl_tile[:, c * 128 : (c + 1) * 128], trp[:]
                            )

                    w_tile = w_pool.tile([128, s * D], mybir.dt.float32, tag="w")
                    nc.gpsimd.dma_start(w_tile[:], w_t[:, p0 * D : (p0 + s) * D])

                    if wide:
                        # One output DMA per pair-window covering all 4 batch
                        # chunks (bigger transfers, better HBM write efficiency).
                        out3 = out.reshape([NCHUNK, 128, PD])
                        for w0 in range(0, s, wide):
                            nw = min(wide, s - w0)
                            stg_tile = stg_pool.tile(
                                [128, NCHUNK * wide * D], mybir.dt.float32, tag="stg"
                            )
                            for c in range(NCHUNK):
                                for q in range(w0, w0 + nw, GROUP):
                                    ng = min(GROUP, w0 + nw - q)
                                    ps = ps_pool.tile(
                                        [128, GROUP * D], mybir.dt.float32, tag="ps"
                                    )
                                    nc.tensor.matmul(
                                        ps[:, : ng * D],
                                        ftl_tile[:, c * 128 : (c + 1) * 128],
                                        w_tile[:, q * D : (q + ng) * D],
                                        start=True,
                                        stop=True,
                                    )
                                    j0 = i + 1 + q
                                    off = (c * nw + (q - w0)) * D
                                    nc.vector.tensor_mul(
                                        stg_tile[:, off : off + ng * D],
                                        ps[:, : ng * D],
                                        fn_tiles[c][:, j0 * D : (j0 + ng) * D],
                                    )
                            out_eng = nc.sync if (i + w0) % 2 == 0 else nc.scalar
                            out_eng.dma_start(
                                out3[:, :, (p0 + w0) * D : (p0 + w0 + nw) * D]
                                .transpose((1, 0, 2)),
                                stg_tile[:, : NCHUNK * nw * D],
                            )
                        p0 += s
                        continue
                    for c in range(NCHUNK):
                        stg_tile = stg_pool.tile([128, s * D], mybir.dt.float32, tag="stg")
                        if ablate == "nocompute":
                            nc.vector.tensor_scalar_mul(
                                stg_tile[:, 0:4], stg_tile[:, 0:4], 0.0
                            )
                        for q in range(0, s, GROUP) if ablate != "nocompute" else []:
                            ng = min(GROUP, s - q)
                            ps = ps_pool.tile([128, GROUP * D], mybir.dt.float32, tag="ps")
                            lhsT = ftl_tile[:, c * 128 : (c + 1) * 128]  # [K=d, M=b]
                            rhs = w_tile[:, q * D : (q + ng) * D]  # [K=d, N=pairs*e]
                            if mm_dt is not None:
                                lhsT = lhsT.bitcast(mm_dt)
                                rhs = rhs.bitcast(mm_dt)
                            nc.tensor.matmul(ps[:, : ng * D], lhsT, rhs, start=True, stop=True)
                            j0 = i + 1 + q
                            nc.vector.tensor_mul(
                                stg_tile[:, q * D : (q + ng) * D],
                                ps[:, : ng * D],
                                fn_tiles[c][:, j0 * D : (j0 + ng) * D],
                            )
                        if ablate != "noout":
                            rings = [nc.sync, nc.scalar, nc.gpsimd][:out_rings]
                            out_eng = rings[(i * NCHUNK + c) % len(rings)]
                            out_eng.dma_start(
                                out[c * 128 : (c + 1) * 128, p0 * D : (p0 + s) * D],
                                stg_tile[:],
                            )
                    p0 += s

    nc.compile()
    return nc


def _wsegs(max_pairs=64):
    """Greedy-pack consecutive i-blocks (sizes F-1-i) into segments of
    <= max_pairs pairs.  Returns list of (i_start, i_end_excl, p0, npairs)."""
    segs = []
    i = 0
    p0 = 0
    while i < F - 1:
        n = 0
        j = i
        while j < F - 1 and n + (F - 1 - j) <= max_pairs:
            n += F - 1 - j
            j += 1
        if j == i:  # single block larger than max_pairs
            n = F - 1 - i
            j = i + 1
        segs.append((i, j, p0, n))
        p0 += n
        i = j
    return segs


def _build_v2(
    niter=1,
    act_every=None,
    rings="sa",  # store ring rotation: s=sync(SP), a=scalar(ACT), g=gpsimd(SWDGE)
    ps_bufs=6,
    stg_bufs=6,
    scr_bufs=6,
    w_bufs=2,
    wseg=64,
    tload="dma",  # "dma": load femb_t from DRAM; "pe": on-chip PE transpose
    gw=4,  # gate-op width in pairs (multiple of GROUP); >4 spans PSUM banks
    ablate=None,  # None | "noout" | "loadsonly" | "storesonly" | "nogate" | "dmaonly"
):
    """v2: merged large input DMAs (femb_n + femb_t fully resident, W in
    ~wseg-pair segment loads), configurable store-ring rotation, deeper
    staging.  Compute structure identical to _build_bf16."""
    if act_every is None:
        act_every = ACT_EVERY
    nc = bacc.Bacc("TRN2", target_bir_lowering=False, debug=False, num_devices=NCORES)
    femb_n = nc.declare_dram_parameter("femb_n", [BSH, FD], mybir.dt.bfloat16, isOutput=False)
    if tload == "dma":
        femb_t = nc.declare_dram_parameter("femb_t", [FD, BSH], mybir.dt.bfloat16, isOutput=False)
    else:
        eye = nc.declare_dram_parameter("eye", [D, D], mybir.dt.bfloat16, isOutput=False)
    w_t = nc.declare_dram_parameter("w_t", [D, PD], mybir.dt.bfloat16, isOutput=False)
    out = nc.declare_dram_parameter("out", [BSH, PD], mybir.dt.bfloat16, isOutput=True)

    import contextlib

    do_loads = ablate not in ("storesonly",)
    do_compute = ablate not in ("loadsonly", "storesonly", "dmaonly")
    do_gate = do_compute and ablate != "nogate"
    do_stores = ablate not in ("noout", "loadsonly")
    segs = _wsegs(wseg)

    with tile.TileContext(nc) as tc:
        with (
            tc.tile_pool(name="eyep", bufs=1) as eye_pool,
            tc.tile_pool(name="fn", bufs=1) as fn_pool,
            tc.tile_pool(name="ftl", bufs=1) as ftl_pool,
            tc.tile_pool(name="w", bufs=w_bufs) as w_pool,
            tc.tile_pool(name="stg", bufs=stg_bufs) as stg_pool,
            tc.tile_pool(name="scr", bufs=scr_bufs) as scr_pool,
            tc.tile_pool(name="ps", bufs=ps_bufs, space="PSUM") as ps_pool,
            tc.tile_pool(name="tr", bufs=2, space="PSUM") as tr_pool,
            tc.For_i(
                0,
                niter,
                1,
                hint_engines=(
                    mybir.EngineType.PE,
                    mybir.EngineType.DVE,
                    mybir.EngineType.Activation,
                    mybir.EngineType.SP,
                ),
            )
            if niter > 1
            else contextlib.nullcontext(),
        ):
            # femb_t fully resident: [d=128, (field, batch)]; slice (i, c) is
            # ftl_all[:, i*BSH + c*128 :][:128].  Loaded in 4 field-groups so
            # the first matmuls don't wait on the whole 3.9 MB.
            ftl_all = ftl_pool.tile([128, F * BSH], mybir.dt.bfloat16, tag="ftl")
            # femb_n fully resident: per-chunk tiles [b=128, (field, emb)].
            fn_tiles = []
            for c in range(NCHUNK):
                fnt = fn_pool.tile([128, FD], mybir.dt.bfloat16, tag=f"fn{c}")
                fn_tiles.append(fnt)
            w_tiles = {}
            if do_loads:
                # first w segment before the bulk femb loads: compute starts
                # after ~2 small DMAs instead of the full 8 MB of femb.
                i0, i1, p0s, npair = segs[0]
                w0_tile = w_pool.tile([128, npair * D], mybir.dt.bfloat16, tag="w")
                nc.gpsimd.dma_start(w0_tile[:], w_t[:, p0s * D : (p0s + npair) * D])
                w_tiles[0] = w0_tile
                if tload == "dma":
                    FGRP = 8  # fields per ftl load group
                    femb_t3 = femb_t.reshape([F, D, BSH])
                    for f0 in range(0, F, FGRP):
                        f1 = min(f0 + FGRP, F)
                        nc.gpsimd.dma_start(
                            ftl_all[:, f0 * BSH : f1 * BSH],
                            femb_t3[f0:f1].transpose((1, 0, 2)),
                        )
                else:
                    eye_tile = eye_pool.tile([D, D], mybir.dt.bfloat16)
                    nc.gpsimd.dma_start(eye_tile[:], eye[:])
                for c in range(NCHUNK):
                    nc.gpsimd.dma_start(
                        fn_tiles[c][:], femb_n[c * 128 : (c + 1) * 128, :]
                    )

            ring_map = {"s": nc.sync, "a": nc.scalar, "g": nc.gpsimd}
            ring_list = [ring_map[ch] for ch in rings]
            grp = 0
            st = 0  # store counter
            for si, (i0, i1, p0s, npair) in enumerate(segs):
                if do_loads and si not in w_tiles:
                    w_seg_tile = w_pool.tile(
                        [128, npair * D], mybir.dt.bfloat16, tag="w"
                    )
                    nc.gpsimd.dma_start(
                        w_seg_tile[:], w_t[:, p0s * D : (p0s + npair) * D]
                    )
                    w_tiles[si] = w_seg_tile
                if si in w_tiles:
                    w_tile = w_tiles[si]
                else:
                    w_tile = w_pool.tile([128, npair * D], mybir.dt.bfloat16, tag="w")
                p0 = p0s
                for i in range(i0, i1):
                    s = F - 1 - i
                    woff = (p0 - p0s) * D
                    if tload == "pe" and do_compute:
                        # build v_i^T [d, b] on-chip: PE transpose (exact data
                        # movement, bf16 PSUM) + ACT copy to the resident tile
                        for c in range(NCHUNK):
                            trp = tr_pool.tile([128, 128], mybir.dt.bfloat16, tag="tr")
                            nc.tensor.transpose(
                                trp[:],
                                fn_tiles[c][:, i * D : (i + 1) * D],
                                eye_tile[:],
                            )
                            nc.scalar.copy(
                                ftl_all[:, i * BSH + c * 128 : i * BSH + (c + 1) * 128],
                                trp[:],
                            )
                    for c in range(NCHUNK):
                        stg_tile = stg_pool.tile(
                            [128, s * D], mybir.dt.bfloat16, tag="stg"
                        )
                        if not do_gate and do_stores:
                            nc.vector.tensor_scalar_mul(
                                stg_tile[:, 0:4], stg_tile[:, 0:4], 0.0
                            )
                        for q in range(0, s, gw) if do_compute else []:
                            ng = min(gw, s - q)
                            ps = ps_pool.tile(
                                [128, gw * D], mybir.dt.float32, tag="ps"
                            )
                            # one matmul per PSUM-bank's worth of pairs
                            for h in range(0, ng, GROUP):
                                nh = min(GROUP, ng - h)
                                nc.tensor.matmul(
                                    ps[:, h * D : (h + nh) * D],
                                    ftl_all[
                                        :, i * BSH + c * 128 : i * BSH + (c + 1) * 128
                                    ],
                                    w_tile[
                                        :,
                                        woff + (q + h) * D : woff + (q + h + nh) * D,
                                    ],
                                    start=True,
                                    stop=True,
                                )
                            j0 = i + 1 + q
                            if not do_gate:
                                grp += 1
                                continue
                            if act_every and grp % act_every != 0:
                                scr = scr_pool.tile(
                                    [128, gw * D], mybir.dt.bfloat16, tag="scr"
                                )
                                nc.scalar.copy(scr[:, : ng * D], ps[:, : ng * D])
                                nc.vector.tensor_mul(
                                    stg_tile[:, q * D : (q + ng) * D],
                                    scr[:, : ng * D],
                                    fn_tiles[c][:, j0 * D : (j0 + ng) * D],
                                )
                            else:
                                nc.vector.tensor_mul(
                                    stg_tile[:, q * D : (q + ng) * D],
                                    ps[:, : ng * D],
                                    fn_tiles[c][:, j0 * D : (j0 + ng) * D],
                                )
                            grp += 1
                        if do_stores:
                            out_eng = ring_list[st % len(ring_list)]
                            st += 1
                            out_eng.dma_start(
                                out[c * 128 : (c + 1) * 128, p0 * D : (p0 + s) * D],
                                stg_tile[:],
                            )
                    p0 += s

    nc.compile()
    return nc


def _build_v3(
    niter=1,
    act_every=3,
    rings="sg",
    pgrp=4,  # pairs per PSUM group (PSUM banks per gate op)
    ps_bufs=None,
    stg_bufs=3,
    scr_bufs=4,
    w_bufs=2,
    wseg=64,
    sblk=16,  # pairs per output store
    ablate=None,  # None | "noout" | "loadsonly" | "storesonly" | "dmaonly" | "nogate"
):
    """v3 (eb layout): per pair p=(i,j), stationary = W_p^T [d, e] slice of the
    resident w segment, moving = v_i^T [d, b] slice of the resident femb_t --
    one N=512 matmul per pair covering the whole per-core batch.  PSUM is
    [e, b]; the gate operand v_j^T is another femb_t slice, so femb_n is never
    loaded.  Output is [P, D, BSH] (host un-transposes).  Gate split between
    direct DVE tensor_mul (fp32 PSUM) and ACT-copy + packed-bf16 DVE path.
    Per-core HBM traffic: 18.2 MB in + 57 MB out."""
    nc = bacc.Bacc("TRN2", target_bir_lowering=False, debug=False, num_devices=NCORES)
    femb_t = nc.declare_dram_parameter("femb_t", [FD, BSH], mybir.dt.bfloat16, isOutput=False)
    w_t = nc.declare_dram_parameter("w_t", [D, PD], mybir.dt.bfloat16, isOutput=False)
    out = nc.declare_dram_parameter("out", [PD, BSH], mybir.dt.bfloat16, isOutput=True)

    import contextlib

    do_loads = ablate not in ("storesonly",)
    do_compute = ablate not in ("loadsonly", "storesonly", "dmaonly")
    do_gate = do_compute and ablate != "nogate"
    do_stores = ablate not in ("noout", "loadsonly")
    segs = _wsegs(wseg)
    if ps_bufs is None:
        ps_bufs = 8 // pgrp

    with tile.TileContext(nc) as tc:
        with (
            tc.tile_pool(name="ftl", bufs=1) as ftl_pool,
            tc.tile_pool(name="w", bufs=w_bufs) as w_pool,
            tc.tile_pool(name="stg", bufs=stg_bufs) as stg_pool,
            tc.tile_pool(name="scr", bufs=scr_bufs) as scr_pool,
            tc.tile_pool(name="ps", bufs=ps_bufs, space="PSUM") as ps_pool,
            tc.For_i(
                0,
                niter,
                1,
                hint_engines=(
                    mybir.EngineType.PE,
                    mybir.EngineType.DVE,
                    mybir.EngineType.Activation,
                    mybir.EngineType.SP,
                ),
            )
            if niter > 1
            else contextlib.nullcontext(),
        ):
            ftl_all = ftl_pool.tile([128, F * BSH], mybir.dt.bfloat16, tag="ftl")
            w_tiles = {}
            if do_loads:
                i0, i1, p0s, npair = segs[0]
                w0_tile = w_pool.tile([128, npair * D], mybir.dt.bfloat16, tag="w")
                nc.gpsimd.dma_start(w0_tile[:], w_t[:, p0s * D : (p0s + npair) * D])
                w_tiles[0] = w0_tile
                FGRP = 8
                femb_t3 = femb_t.reshape([F, D, BSH])
                for f0 in range(0, F, FGRP):
                    f1 = min(f0 + FGRP, F)
                    nc.gpsimd.dma_start(
                        ftl_all[:, f0 * BSH : f1 * BSH],
                        femb_t3[f0:f1].transpose((1, 0, 2)),
                    )

            # e-major output layout [D, P, BSH]: a store of nb consecutive
            # pairs is nb*BSH*2 bytes CONTIGUOUS per partition (16-32 KB
            # descriptors -> line-rate HBM writes).  Host un-transposes.
            out3 = out.reshape([D, P, BSH])
            ring_map = {"s": nc.sync, "a": nc.scalar, "g": nc.gpsimd}
            ring_list = [ring_map[ch] for ch in rings]
            grp = 0
            st = 0
            for si, (i0, i1, p0s, npair) in enumerate(segs):
                if do_loads and si not in w_tiles:
                    w_seg_tile = w_pool.tile(
                        [128, npair * D], mybir.dt.bfloat16, tag="w"
                    )
                    nc.gpsimd.dma_start(
                        w_seg_tile[:], w_t[:, p0s * D : (p0s + npair) * D]
                    )
                    w_tiles[si] = w_seg_tile
                if si in w_tiles:
                    w_tile = w_tiles[si]
                else:
                    w_tile = w_pool.tile([128, npair * D], mybir.dt.bfloat16, tag="w")
                p0 = p0s
                for i in range(i0, i1):
                    s = F - 1 - i
                    woff = (p0 - p0s) * D
                    for b0 in range(0, s, sblk):
                        nb = min(sblk, s - b0)
                        stg_tile = stg_pool.tile(
                            [128, sblk * BSH], mybir.dt.bfloat16, tag="stg"
                        )
                        if not do_gate and do_stores:
                            nc.vector.tensor_scalar_mul(
                                stg_tile[:, 0:4], stg_tile[:, 0:4], 0.0
                            )
                        for q in range(b0, b0 + nb, pgrp) if do_compute else []:
                            npq = min(pgrp, b0 + nb - q)
                            ps = ps_pool.tile(
                                [128, pgrp * BSH], mybir.dt.float32, tag="ps"
                            )
                            for h in range(npq):
                                nc.tensor.matmul(
                                    ps[:, h * BSH : (h + 1) * BSH],
                                    w_tile[
                                        :, woff + (q + h) * D : woff + (q + h + 1) * D
                                    ],  # [K=d, M=e] stationary
                                    ftl_all[:, i * BSH : (i + 1) * BSH],  # [K=d, N=b]
                                    start=True,
                                    stop=True,
                                )
                            j0 = i + 1 + q
                            gate = ftl_all[:, j0 * BSH : (j0 + npq) * BSH]
                            so = (q - b0) * BSH
                            if not do_gate:
                                grp += 1
                                continue
                            if act_every and grp % act_every != 0:
                                scr = scr_pool.tile(
                                    [128, pgrp * BSH], mybir.dt.bfloat16, tag="scr"
                                )
                                nc.scalar.copy(
                                    scr[:, : npq * BSH], ps[:, : npq * BSH]
                                )
                                nc.vector.tensor_mul(
                                    stg_tile[:, so : so + npq * BSH],
                                    scr[:, : npq * BSH],
                                    gate,
                                )
                            else:
                                nc.vector.tensor_mul(
                                    stg_tile[:, so : so + npq * BSH],
                                    ps[:, : npq * BSH],
                                    gate,
                                )
                            grp += 1
                        if do_stores:
                            out_eng = ring_list[st % len(ring_list)]
                            st += 1
                            out_eng.dma_start(
                                out3[:, p0 + b0 : p0 + b0 + nb, :],
                                stg_tile[:, : nb * BSH],
                            )
                    p0 += s

    nc.compile()
    return nc


def _input_names(nc):
    names = set()
    for alloc in nc.m.functions[0].allocations:
        if isinstance(alloc, mybir.MemoryLocationSet) and alloc.kind == "ExternalInput":
            names.add(alloc.memorylocations[0].name)
    return names


def _prep_in_maps(femb, Wc, names):
    """Per-core input maps (full-precision host arrays -> device layouts)."""
    # w_t[d, p*D + e] = W[p, e, d]
    w_t = np.ascontiguousarray(Wc.transpose(2, 0, 1)).reshape(D, PD)
    eye = np.eye(D, dtype=np.float32)
    if MODE in ("bf16", "v2", "v3"):
        femb = femb.astype(BF16)
        w_t = w_t.astype(BF16)
        eye = eye.astype(BF16)
    ft_all = femb.transpose(1, 2, 0)  # [F, D, B] view
    in_maps = []
    for co in range(NCORES):
        sl = slice(co * BSH, (co + 1) * BSH)
        m = {
            "femb_n": femb[sl].reshape(BSH, FD),
            "femb_t": np.ascontiguousarray(ft_all[:, :, sl]).reshape(FD, BSH),
            "w_t": w_t,
            "eye": eye,
        }
        in_maps.append({k: v for k, v in m.items() if k in names})
    return in_maps


def _default_builder(niter=1):
    if MODE == "v3":
        return _build_v3(niter=niter, **V2_KW)
    if MODE == "v2":
        return _build_v2(niter=niter, **V2_KW)
    if MODE == "bf16":
        return _build_bf16(niter=niter)
    return _build(niter=niter, mode=MODE)


V2_KW = {"pgrp": 2, "sblk": 24}


def _get_nc():
    key = (MODE, tuple(sorted(V2_KW.items())))
    if _cache.get("mode") != key:
        _cache["nc"] = _default_builder()
        _cache["mode"] = key
    return _cache["nc"]


def kernel(feature_emb, W):
    global last_results
    femb = np.ascontiguousarray(feature_emb, dtype=np.float32)
    Wc = np.asarray(W, dtype=np.float32)
    assert femb.shape == (B, F, D) and Wc.shape == (P, D, D)

    nc = _get_nc()
    in_maps = _prep_in_maps(femb, Wc, _input_names(nc))

    res = run_bass_kernel_spmd(nc, in_maps, list(range(NCORES)), trace=TRACE)
    last_results = res

    out = np.empty((B, P, D), dtype=np.float32)
    for co in range(NCORES):
        o = res.results[co]["out"]
        if MODE == "v3":
            out[co * BSH : (co + 1) * BSH] = (
                np.asarray(o, dtype=np.float32).reshape(D, P, BSH).transpose(2, 1, 0)
            )
        elif MODE == "eb":
            out[co * BSH : (co + 1) * BSH] = (
                np.asarray(o, dtype=np.float32).reshape(P, D, BSH).transpose(2, 0, 1)
            )
        else:
            out[co * BSH : (co + 1) * BSH] = np.asarray(
                o, dtype=np.float32
            ).reshape(BSH, P, D)
    return out


# ---------------------------------------------------------------------------
# Timing support (used by test.py; not needed for grading correctness).
# The local axon build has no NTFF profile hook, so HW time is measured as the
# marginal wall-clock of an in-NEFF repeat loop with device-resident inputs:
# t(niter=N) - t(niter=1) cancels all host/tunnel/launch constants.
# ---------------------------------------------------------------------------


def _make_runner(nc, n_cores=NCORES):
    import jax
    import jax.numpy as jnp
    from jax.sharding import Mesh, NamedSharding, PartitionSpec
    from jax.experimental.shard_map import shard_map

    from concourse import bass2jax

    bass2jax.install_neuronx_cc_hook()
    partition_name = nc.partition_id_tensor.name if nc.partition_id_tensor else None
    in_names, out_names, out_avals = [], [], []
    for alloc in nc.m.functions[0].allocations:
        if not isinstance(alloc, mybir.MemoryLocationSet):
            continue
        name = alloc.memorylocations[0].name
        if alloc.kind == "ExternalInput":
            if name != partition_name:
                in_names.append(name)
        elif alloc.kind == "ExternalOutput":
            out_names.append(name)
            out_avals.append(
                jax.core.ShapedArray(tuple(alloc.tensor_shape), mybir.dt.np(alloc.dtype))
            )
    n_params, n_outs = len(in_names), len(out_names)
    all_names = in_names + out_names + ([partition_name] if partition_name else [])

    def _body(*args):
        operands = list(args)
        if partition_name is not None:
            operands.append(bass2jax.partition_id_tensor())
        return tuple(
            bass2jax._bass_exec_p.bind(
                *operands,
                out_avals=tuple(out_avals),
                in_names=tuple(all_names),
                out_names=tuple(out_names),
                lowering_input_output_aliases=(),
                sim_require_finite=True,
                sim_require_nnan=True,
                nc=nc,
            )
        )

    mesh = Mesh(np.asarray(jax.devices()[:n_cores]), ("core",))
    spec = PartitionSpec("core")
    sharded = jax.jit(
        shard_map(
            _body,
            mesh=mesh,
            in_specs=(spec,) * (n_params + n_outs),
            out_specs=(spec,) * n_outs,
            check_rep=False,
        ),
        donate_argnums=tuple(range(n_params, n_params + n_outs)),
        keep_unused=True,
    )
    sharding = NamedSharding(mesh, spec)
    zeros_fn = jax.jit(
        lambda: tuple(
            jnp.zeros((n_cores * a.shape[0], *a.shape[1:]), a.dtype) for a in out_avals
        ),
        out_shardings=(sharding,) * n_outs,
    )
    return sharded, zeros_fn, in_names, sharding


def _bench_once(niter, in_maps, reps=4, builder=None):
    import time

    import jax

    nc = builder(niter=niter) if builder is not None else _default_builder(niter=niter)
    sharded, zeros_fn, in_names, sharding = _make_runner(nc)
    dev_in = [
        jax.device_put(np.concatenate([m[n] for m in in_maps], axis=0), sharding)
        for n in in_names
    ]
    for a in dev_in:
        a.block_until_ready()
    times = []
    for _ in range(reps):
        zeros = zeros_fn()
        for z in zeros:
            z.block_until_ready()
        t0 = time.time()
        outs = sharded(*dev_in, *zeros)
        for o in outs:
            o.block_until_ready()
        times.append(time.time() - t0)
    return min(times)


def measure_hw_time_ns(feature_emb, W, niter=201, reps=6):
    """Marginal per-iteration HW time of the kernel NEFF, in ns."""
    femb = np.ascontiguousarray(feature_emb, dtype=np.float32)
    Wc = np.asarray(W, dtype=np.float32)
    names = _input_names(_get_nc())
    in_maps = _prep_in_maps(femb, Wc, names)
    t1 = _bench_once(1, in_maps, reps)
    tn = _bench_once(niter, in_maps, reps)
    return (tn - t1) / (niter - 1) * 1e9, t1, tn

